# revision 13
# baseline (speedup 1.0000x reference)
"""Trainium2 Bass kernel for complex multi-head attention (8 NeuronCores).

Sharding: core c handles batch b = c//2 and head-group g = c%2 (8 of 16
heads, i.e. 512 of 1024 embed dims). No device collectives: each core
produces a partial out-projection (its head-group's contribution) and the
host sums the two partials per batch and adds the output bias.

Per-core dataflow (all matmuls bf16 with fp32 PSUM accumulation):
  - V projected first (natural layout) into VRIO[s, st, h, d] = [vr | vi],
    then K projected transposed into stacked score operands KA = [kr; ki],
    KB = [ki; -kr]. Q projections (R = [qr; qi]) are interleaved per-dt
    chunk with the attention pipeline so their PE time hides under the
    softmax's ACT/DVE/GPSIMD work.
  - Scores: one K=128 matmul pair per (kc, qh) tile yields attn_real^T and
    -attn_imag^T with q on the free axis. s2 = re^2 + im^2 is built with
    per-tile routing measured on HW: half the tiles ACT Square (1.11us,
    reads PSUM directly), half DVE copy+bf16 self-multiply (1.22+0.62us,
    DVE 2x mode); the halves-add runs on GPSIMD (its only fast-enough
    role). Per-head batched ACT sqrt (folding SCALE^2) + in-place exp over
    [128, 8*1024]. No max-subtraction: exp args are <= ~4 here.
  - AV matmuls produce att^T directly (lhsT = V tile, rhs = probs tile) -
    no PE transpose phase. Denominators: DVE tree-adds the 8 probs tiles
    (bf16 2x), a [k,1]-ones matmul reduces partitions, gpsimd
    partition_broadcast + DVE reciprocal, folded into the AV PSUM
    evacuation multiply.
  - Output projection accumulates over the 4 head-pair chunks; out weights
    are DMA'd late into the idle input-stream buffers.
"""

import os
import sys

for _p in ("/opt/trn_rl_repo", "/root/.axon_site/_ro/trn_rl_repo"):
    if os.path.isdir(_p) and _p not in sys.path:
        sys.path.append(_p)

import numpy as np
import ml_dtypes

bf16 = ml_dtypes.bfloat16

P = 128
S = 1024
E = 1024
DL = 512  # local (per-core) head dims: 8 heads x 64
D = 64
HLOC = 8
SCALE = D ** -0.5

_NC_CACHE = None


def _build():
    import concourse.tile as tile
    from concourse import bacc, mybir

    f32 = mybir.dt.float32
    b16 = mybir.dt.bfloat16
    Alu = mybir.AluOpType
    Act = mybir.ActivationFunctionType

    nc = bacc.Bacc("TRN2", target_bir_lowering=False, debug=False, num_devices=8)

    def din(name, shape, dt):
        return nc.dram_tensor(name, shape, dt, kind="ExternalInput").ap()

    x_in = {n: din(n, [E, S], b16)
            for n in ("xq_r", "xq_i", "xk_r", "xk_i", "xv_r", "xv_i")}
    w_in = {n: din(n, [E, DL], b16)
            for n in ("wq_r", "wq_i", "wk_r", "wk_i", "wv_r", "wv_i")}
    wo_in = {n: din(n, [DL, E], b16) for n in ("wo_r", "wo_i")}
    bqk_in = {n: din(n, [P, 4], f32)
              for n in ("bq_r", "bq_i", "bk_r", "bk_i")}
    bv_in = {n: din(n, [P, DL], f32) for n in ("bv_r", "bv_i")}
    out_d = {n: nc.dram_tensor(n, [S, E], f32, kind="ExternalOutput").ap()
             for n in ("out_r", "out_i")}

    with tile.TileContext(nc) as tc:
        with (
            tc.tile_pool(name="persist", bufs=1) as persist,
            tc.tile_pool(name="stream", bufs=2) as stream,
            tc.tile_pool(name="pssc", bufs=2, space="PSUM") as pssc,
        ):
            R_st = persist.tile([P, HLOC, S], b16, tag="R")
            KA_st = persist.tile([P, HLOC, S], b16, tag="KA")
            KB_st = persist.tile([P, HLOC, S], b16, tag="KB")
            VRIO = persist.tile([P, 8, HLOC, P], b16, tag="VRIO")
            AttTr = persist.tile([P, 4, S], b16, tag="AttTr")
            AttTi = persist.tile([P, 4, S], b16, tag="AttTi")
            ones = persist.tile([P, 1], b16, tag="ones")
            nc.vector.memset(ones[:], 1.0)

            bias_sb = {}
            for n, ap in bqk_in.items():
                t = persist.tile([P, 4], f32, tag=n)
                nc.sync.dma_start(t[:], ap)
                bias_sb[n] = t
            for n, ap in bv_in.items():
                t = persist.tile([P, DL], f32, tag=n)
                nc.sync.dma_start(t[:], ap)
                bias_sb[n] = t

            def load_xw(xn, wn):
                x_sb = stream.tile([P, 8, S], b16, tag="x")
                nc.sync.dma_start(
                    x_sb[:], x_in[xn].rearrange("(eo p) s -> p eo s", p=P))
                w_sb = stream.tile([P, 8, DL], b16, tag="w")
                nc.sync.dma_start(
                    w_sb[:], w_in[wn].rearrange("(eo p) d -> p eo d", p=P))
                return x_sb, w_sb

            # ---- Phase 1: V projections (natural out) into VRIO
            for xn, wn, bn, ri in (("xv_r", "wv_r", "bv_r", 0),
                                   ("xv_i", "wv_i", "bv_i", 1)):
                x_sb, w_sb = load_xw(xn, wn)
                for st in range(8):
                    ps = pssc.tile([P, S], f32, tag="sc")
                    for eo in range(8):
                        nc.tensor.matmul(
                            ps[:, 0:DL], x_sb[:, eo, st * P:(st + 1) * P],
                            w_sb[:, eo, :],
                            start=(eo == 0), stop=(eo == 7))
                    nc.vector.tensor_tensor(
                        VRIO[:, st, :, ri * D:(ri + 1) * D],
                        ps[:, 0:DL].rearrange("p (h d) -> p h d", h=HLOC),
                        bias_sb[bn][:].rearrange("p (h d) -> p h d", h=HLOC),
                        Alu.add)

            # ---- Phase 2: K (full) then Q (per-dt, interleaved) projections
            k_projs = [
                ("xk_r", "wk_r", [(KA_st, 0, "bk_r", 1.0),
                                  (KB_st, 64, "bk_r", -1.0)]),
                ("xk_i", "wk_i", [(KA_st, 64, "bk_i", 1.0),
                                  (KB_st, 0, "bk_i", 1.0)]),
            ]
            q_projs = [
                ("xq_r", "wq_r", [(R_st, 0, "bq_r", 1.0)]),
                ("xq_i", "wq_i", [(R_st, 64, "bq_i", 1.0)]),
            ]

            def proj_dt(x_sb, w_sb, evacs, dt):
                ps = pssc.tile([P, S], f32, tag="sc")
                for eo in range(8):
                    for nh in range(2):
                        nc.tensor.matmul(
                            ps[:, nh * 512:(nh + 1) * 512],
                            w_sb[:, eo, dt * P:(dt + 1) * P],
                            x_sb[:, eo, nh * 512:(nh + 1) * 512],
                            start=(eo == 0), stop=(eo == 7))
                for half in range(2):
                    h = 2 * dt + half
                    src = ps[64 * half:64 * half + 64, :]
                    for dest, base, bn, sc in evacs:
                        b_ap = bias_sb[bn][64 * half:64 * half + 64, dt:dt + 1]
                        dst = dest[base:base + 64, h, :]
                        if sc == 1.0:
                            nc.vector.tensor_scalar(
                                dst, src, b_ap, None, op0=Alu.add)
                        else:
                            nc.vector.tensor_scalar(
                                dst, src, b_ap, sc, op0=Alu.add, op1=Alu.mult)

            for xn, wn, evacs in k_projs:
                x_sb, w_sb = load_xw(xn, wn)
                for dt in range(4):
                    proj_dt(x_sb, w_sb, evacs, dt)

            qx = {}
            for xn, wn, evacs in q_projs:
                qx[xn] = (*load_xw(xn, wn), evacs)

            # ---- Phase 3: attention per head, Q projections interleaved
            with (
                tc.tile_pool(name="s2p", bufs=2) as s2p,
                tc.tile_pool(name="sqp", bufs=3) as sqp,
                tc.tile_pool(name="trp", bufs=2) as trp,
                tc.tile_pool(name="recp", bufs=2) as recp,
                tc.tile_pool(name="psav", bufs=1, space="PSUM") as psav,
            ):
                s2_tiles = {}

                def emit_scores(h):
                    s2 = s2p.tile([P, HLOC, S], b16, tag="s2")
                    s2_tiles[h] = s2
                    for kc in range(8):
                        for qh in range(2):
                            ps = pssc.tile([P, S], f32, tag="sc")
                            nc.tensor.matmul(
                                ps[:, 0:512],
                                KA_st[:, h, kc * P:(kc + 1) * P],
                                R_st[:, h, qh * 512:(qh + 1) * 512],
                                start=True, stop=True)
                            nc.tensor.matmul(
                                ps[:, 512:1024],
                                KB_st[:, h, kc * P:(kc + 1) * P],
                                R_st[:, h, qh * 512:(qh + 1) * 512],
                                start=True, stop=True)
                            # routed square: half ACT (direct from PSUM),
                            # half DVE copy + bf16 2x self-multiply
                            sq = sqp.tile([P, S], b16, tag="sq")
                            if kc % 2 == 0:
                                nc.scalar.activation(sq[:], ps[:], Act.Square)
                            else:
                                cp = sqp.tile([P, S], b16, tag="cp")
                                nc.vector.tensor_scalar(
                                    cp[:], ps[:], 0.0, None, op0=Alu.add)
                                nc.vector.tensor_tensor(
                                    sq[:], cp[:], cp[:], Alu.mult)
                            nc.gpsimd.tensor_tensor(
                                s2[:, kc, qh * 512:(qh + 1) * 512],
                                sq[:, 0:512], sq[:, 512:1024], Alu.add)

                def emit_softmax_av(h):
                    s2 = s2_tiles.pop(h)
                    # mag*SCALE = sqrt(s2 * SCALE^2); then in-place exp.
                    nc.scalar.activation(
                        s2[:], s2[:], Act.Sqrt, scale=float(SCALE) ** 2)
                    nc.scalar.activation(s2[:], s2[:], Act.Exp)
                    # denominator: bf16-2x tree-add of the 8 kc tiles, then
                    # a [k,1]-ones matmul reduces the 128 partitions.
                    t4 = [trp.tile([P, S], b16, tag=f"t{j}", name=f"t{j}")
                          for j in range(4)]
                    for j in range(4):
                        nc.vector.tensor_tensor(
                            t4[j][:], s2[:, 2 * j, :], s2[:, 2 * j + 1, :],
                            Alu.add)
                    nc.vector.tensor_tensor(t4[0][:], t4[0][:], t4[1][:],
                                            Alu.add)
                    nc.vector.tensor_tensor(t4[2][:], t4[2][:], t4[3][:],
                                            Alu.add)
                    sumt = t4[0]
                    nc.vector.tensor_tensor(sumt[:], t4[0][:], t4[2][:],
                                            Alu.add)
                    # AV + den share one PSUM tile: av in [0:1024],
                    # den row 0 in [1024:2048]
                    pav = psav.tile([P, 2 * S], f32, tag="av")
                    for qh in range(2):
                        nc.tensor.matmul(
                            pav[0:1, S + qh * 512:S + (qh + 1) * 512],
                            ones[:, 0:1],
                            sumt[:, qh * 512:(qh + 1) * 512],
                            start=True, stop=True)
                    den1 = recp.tile([1, S], b16, tag="den1")
                    nc.vector.tensor_scalar(
                        den1[:], pav[0:1, S:2 * S], 0.0, None, op0=Alu.add)
                    den = recp.tile([P, S], b16, tag="den")
                    nc.gpsimd.partition_broadcast(den[:], den1[:])
                    rec = recp.tile([P, S], b16, tag="rec")
                    with nc.allow_low_precision(
                            reason="softmax denom scale, 0.4% ok"):
                        nc.vector.reciprocal(rec[:], den[:])
                    # AV: att^T accumulation over kc chunks
                    for qh in range(2):
                        for kc in range(8):
                            nc.tensor.matmul(
                                pav[:, qh * 512:(qh + 1) * 512],
                                VRIO[:, kc, h, :],
                                s2[:, kc, qh * 512:(qh + 1) * 512],
                                start=(kc == 0), stop=(kc == 7))
                    # evacuate normalized att^T halves into AttTr/AttTi
                    rb = 64 * (h % 2)
                    nc.vector.tensor_tensor(
                        AttTr[rb:rb + 64, h // 2, :], pav[0:64, 0:S],
                        rec[0:64, :], Alu.mult)
                    nc.vector.tensor_tensor(
                        AttTi[rb:rb + 64, h // 2, :], pav[64:128, 0:S],
                        rec[64:128, :], Alu.mult)

                for dt in range(4):
                    for xn in ("xq_r", "xq_i"):
                        x_sb, w_sb, evacs = qx[xn]
                        proj_dt(x_sb, w_sb, evacs, dt)
                    for h in (2 * dt, 2 * dt + 1):
                        emit_scores(h)
                        if h >= 1:
                            emit_softmax_av(h - 1)
                emit_softmax_av(HLOC - 1)

                # wo loads reuse the stream x slots (idle after Q projs)
                wo_sb = {}
                for n, ap in wo_in.items():
                    t = stream.tile([P, 8, S], b16, tag="x")
                    nc.sync.dma_start(
                        t[:, 0:4, :], ap.rearrange("(dc p) o -> p dc o", p=P))
                    wo_sb[n] = t

            # ---- Phase 4: output projections (partial, no bias)
            with tc.tile_pool(name="fin", bufs=4) as fin:
                for att, wn, on in ((AttTr, "wo_r", "out_r"),
                                    (AttTi, "wo_i", "out_i")):
                    for st in range(8):
                        for oh in range(2):
                            ps = pssc.tile([P, S], f32, tag="sc")
                            for dc in range(4):
                                nc.tensor.matmul(
                                    ps[:, 0:512],
                                    att[:, dc, st * P:(st + 1) * P],
                                    wo_sb[wn][:, dc, oh * 512:(oh + 1) * 512],
                                    start=(dc == 0), stop=(dc == 3))
                            ob = fin.tile([P, 512], f32, tag="ob")
                            nc.vector.tensor_copy(ob[:], ps[:, 0:512])
                            nc.sync.dma_start(
                                out_d[on][st * P:(st + 1) * P,
                                          oh * 512:(oh + 1) * 512], ob[:])

    nc.compile()
    return nc


def make_in_maps(inputs):
    """Shard + host-prep the full inputs into 8 per-core input maps."""
    inp = {k: np.asarray(v) for k, v in inputs.items()}
    xs = {
        "xq": ("query_real", "query_imag"),
        "xk": ("key_real", "key_imag"),
        "xv": ("value_real", "value_imag"),
    }
    per_g = []
    for g in range(2):
        rows = slice(g * DL, (g + 1) * DL)
        m = {}
        for wn, src in (("wq_r", "Wq_r"), ("wq_i", "Wq_i"),
                        ("wk_r", "Wk_r"), ("wk_i", "Wk_i"),
                        ("wv_r", "Wv_r"), ("wv_i", "Wv_i")):
            m[wn] = np.ascontiguousarray(inp[src][rows].T).astype(bf16)
        for wn, src in (("wo_r", "Wo_r"), ("wo_i", "Wo_i")):
            m[wn] = np.ascontiguousarray(inp[src][:, rows].T).astype(bf16)
        for bn, src in (("bq_r", "bq_r"), ("bq_i", "bq_i"),
                        ("bk_r", "bk_r"), ("bk_i", "bk_i")):
            m[bn] = np.ascontiguousarray(
                inp[src][rows].reshape(4, P).T).astype(np.float32)
        for bn, src in (("bv_r", "bv_r"), ("bv_i", "bv_i")):
            m[bn] = np.ascontiguousarray(
                np.broadcast_to(inp[src][rows], (P, DL))).astype(np.float32)
        per_g.append(m)

    in_maps = []
    for c in range(8):
        b, g = c // 2, c % 2
        m = dict(per_g[g])
        for pref, (re_n, im_n) in xs.items():
            m[pref + "_r"] = np.ascontiguousarray(inp[re_n][b].T).astype(bf16)
            m[pref + "_i"] = np.ascontiguousarray(inp[im_n][b].T).astype(bf16)
        in_maps.append(m)
    return in_maps


def combine_outputs(results, inputs):
    bo_r = np.asarray(inputs["bo_r"], np.float32)
    bo_i = np.asarray(inputs["bo_i"], np.float32)
    B = 4
    out_r = np.empty((B, S, E), np.float32)
    out_i = np.empty((B, S, E), np.float32)
    for b in range(B):
        out_r[b] = results[2 * b]["out_r"] + results[2 * b + 1]["out_r"] + bo_r
        out_i[b] = results[2 * b]["out_i"] + results[2 * b + 1]["out_i"] + bo_i
    return out_r, out_i


def get_nc():
    global _NC_CACHE
    if _NC_CACHE is None:
        _NC_CACHE = _build()
    return _NC_CACHE


def kernel(**inputs):
    from concourse.bass_utils import run_bass_kernel_spmd

    nc = get_nc()
    in_maps = make_in_maps(inputs)
    res = run_bass_kernel_spmd(nc, in_maps, list(range(8)))
    return combine_outputs(res.results, inputs)
